# revision 55
# baseline (speedup 1.0000x reference)
"""CrossAttention Trainium2 Bass kernel.

Problem: B=2, Q=S=2048, D=1024, H=16 heads, A=64 head_dim.
  q = (iQ @ Wq)   -> [B,H,Q,A]
  k,v = iK @ Wkv  -> [B,H,S,A] each
  scores = q k^T / 8, mask -> -1e9, softmax over S
  out = (attn @ v) @ Wo -> [B,Q,D]

Sharding: 8 cores = 2 batches x 4 head-groups (4 heads each).
Each core computes a partial [Q, D] = ctx_local @ Wo_rows(local heads);
host sums the 4 partials per batch (row-parallel Wo unshard).

Mask pruning: masked s positions contribute exactly 0 to the softmax
(exp(-1e9) == 0.0 in f32, same as the reference), so the host gathers
only the unmasked iK rows (~S/2 of them), padded to a multiple of 128
with zero K columns and -1e9 bias so padding also exps to exactly 0.

Device layout trick: everything is computed "transposed" (feature dim on
partitions) so no on-device transposes are needed:
  - host ships iQ^T, iK^T (pre-tiled [128, 8, n])
  - qT[a,q], kT[a,s] from matmul(lhsT=W, rhs=iX^T)
  - scoresT[s,q] = matmul(lhsT=kT_slice, rhs=qT)       (K=64 contraction)
  - exp via scalar activation, mask bias is a per-partition bias AP
  - V kept natural [s,a] with an appended ones column -> attn@V matmul
    also yields the softmax denominator row for free

Schedule (the attention phase is paced by the ACT engine's exps --
72 x ~1.1us -- and the PE's p-state only ramps to full clock under
continuous execution, so the PE stream is kept dense):
  - DMA order = consumption order: wk, ikt, wq, wv, mb, iqt(c0),
    iqt(c1), wo.  iqt is split into two SBUF tiles so chunk-0 Q proj
    doesn't wait on the chunk-1 transfer.
  - PE: K proj -> V proj -> Q proj(c0) -> attention qc0 (Q proj c1
    matmuls spread between heads as filler) -> attention qc1 (Wo for
    qc0's 8 q-tiles spread between heads) -> tail Wo for qc1.
  - normalize: reciprocal_approx_fast on DVE (1.3us, vs 6.5us exact),
    gpsimd partition_broadcast, DVE multiply straight out of PSUM.
    ctx PSUM is double-buffered so the next head never waits on it.
  - Wo psum->sbuf copies run on DVE mid-phase (ACT is the pacer there)
    and alternate ACT/DVE in the tail (ACT is idle there).
"""

import sys
import numpy as np

for _p in ("/opt/trn_rl_repo",):
    if _p not in sys.path:
        sys.path.insert(0, _p)

import ml_dtypes

B, Q, S, D = 2, 2048, 2048, 1024
H, A = 16, 64
HG = 4            # heads per core
NCORES = 8
NEG = -1e9
MIN_NST = 9       # S tiles after mask pruning (1152 slots; count ~1024)

_cache = {}


def _build_program(nst):
    import concourse.bass as bass  # noqa
    import concourse.bacc as bacc
    import concourse.tile as tile
    from concourse import mybir

    f32 = mybir.dt.float32
    bf16 = mybir.dt.bfloat16
    EXP = mybir.ActivationFunctionType.Exp
    COPY = mybir.ActivationFunctionType.Copy
    LN = mybir.ActivationFunctionType.Ln
    MULT = mybir.AluOpType.mult

    nc = bacc.Bacc("TRN2", target_bir_lowering=False, debug=False)

    SP = nst * 128  # padded kept-S extent (nst is a multiple of 3)
    NSC_ = nst // 3
    iqt = nc.dram_tensor("iqt", [128, 8, Q], bf16, kind="ExternalInput").ap()
    # chunk-major so each s-chunk is one contiguous-per-partition transfer
    ikt = nc.dram_tensor(
        "ikt", [128, NSC_, 8, 384], bf16, kind="ExternalInput"
    ).ap()
    wq = nc.dram_tensor("wq", [128, 8, 256], bf16, kind="ExternalInput").ap()
    wk = nc.dram_tensor("wk", [128, 8, 256], bf16, kind="ExternalInput").ap()
    wv = nc.dram_tensor("wv", [128, 8, 256], bf16, kind="ExternalInput").ap()
    wo = nc.dram_tensor("wo", [128, 2, D], bf16, kind="ExternalInput").ap()
    mb = nc.dram_tensor("mb", [128, nst], f32, kind="ExternalInput").ap()
    out = nc.dram_tensor("out", [128, 16, D], bf16, kind="ExternalOutput").ap()

    NDT = D // 128          # 8 d tiles

    with tile.TileContext(nc) as tc:
        with (
            tc.tile_pool(name="persist", bufs=1) as persist,
            tc.tile_pool(name="expp", bufs=4) as expp,
            tc.tile_pool(name="outp", bufs=3) as outp,
            tc.tile_pool(name="srp", bufs=1) as srp,
            tc.tile_pool(name="bcp", bufs=3) as bcp,
            tc.tile_pool(name="rsp", bufs=2) as rsp,
            tc.tile_pool(name="scp", bufs=3, space="PSUM") as scp,
            tc.tile_pool(name="ctxp", bufs=1, space="PSUM") as ctxp,
        ):
            # ---- persistent tiles ----
            wk_sb = persist.tile([128, 8, 256], bf16, tag="wk")
            wq_sb = persist.tile([128, 8, 256], bf16, tag="wq")
            wv_sb = persist.tile([128, 8, 256], bf16, tag="wv")
            wo_sb = persist.tile([128, 2, D], bf16, tag="wo")
            # ikt split into s-major chunks (3 s-tiles each) so the K/V
            # projections start as soon as the first chunk lands instead of
            # waiting for the whole 2.25MB transfer.
            NSC = nst // 3
            ikt_sc = [
                persist.tile(
                    [128, 8, 384], bf16, tag=f"ikt{sc}", name=f"ikt{sc}",
                )
                for sc in range(NSC)
            ]
            # iqt split per q-chunk so chunk-0 consumers don't wait on the
            # chunk-1 DMA (tile-granular dependency tracking).
            iqt_c = [
                persist.tile([128, 8, 1024], bf16, tag=f"iqt{c}", name=f"iqt{c}")
                for c in range(2)
            ]
            mb_sb = persist.tile([128, nst], f32, tag="mb")
            # qt split per q-chunk: chunk-1 projection lands mid-attention
            # and must not create a (tile-granular) false dependency for
            # chunk-0 score reads.
            qt_c = [
                persist.tile([128, 2, 1024], bf16, tag=f"qt{c}", name=f"qt{c}")
                for c in range(2)
            ]
            kt_sb = persist.tile([128, 2, SP], bf16, tag="kt")
            # V padded to 128 cols (fast weight load wants full-width lhsT);
            # col 64 = ones (softmax denominator row), cols 65.. = zeros.
            v_sb = persist.tile([128, nst, HG, 128], bf16, tag="v")
            # two heads packed per 128-partition tile for the Wo matmul;
            # separate tiles per q-chunk so Wo reads of chunk 0 don't
            # falsely depend on chunk 1 writes.
            ctxn = [
                [
                    persist.tile(
                        [128, 1024], bf16, tag=f"ctxn{qc}{t}", name=f"ctxn{qc}{t}"
                    )
                    for t in range(2)
                ]
                for qc in range(2)
            ]

            # ---- DMAs in consumption order, consolidated (one issue per
            # tensor chunk costs ~620ns of queue time) and split across
            # the two HWDGE-capable queues (sync + scalar).
            nc.scalar.dma_start(wk_sb[:], wk[:])
            nc.scalar.dma_start(wv_sb[:], wv[:])
            for sc in range(NSC):
                nc.sync.dma_start(ikt_sc[sc][:], ikt[:, sc, :, :])
            nc.scalar.dma_start(wq_sb[:], wq[:])
            nc.scalar.dma_start(mb_sb[:], mb[:])
            for c in range(2):
                nc.scalar.dma_start(
                    iqt_c[c][:], iqt[:, :, c * 1024:(c + 1) * 1024]
                )
            nc.sync.dma_start(wo_sb[:], wo[:])

            # v_sb zero/ones init on gpsimd (idle early; DVE stays free).
            nc.gpsimd.memset(v_sb[:], 0.0)
            nc.gpsimd.memset(v_sb[:, :, :, 64:65], 1.0)

            # ---- K projection: kT [a, s], both 128-row at-slabs ----
            # copies alternate ACT/DVE so neither serializes the chain.
            cp_flip = [0]

            def psum_copy(dst, src):
                if cp_flip[0] % 2 == 0:
                    nc.vector.tensor_copy(out=dst, in_=src)
                else:
                    nc.scalar.activation(out=dst, in_=src, func=COPY)
                cp_flip[0] += 1

            # K and V projections interleaved per s-chunk: each chunk's
            # matmuls start as soon as its 8 ikt d-tiles land.
            for sc in range(NSC):
                w = ikt_sc[sc].shape[2]
                for at in range(2):
                    ps = scp.tile([128, 384], f32, tag="mm")
                    for dt_i in range(NDT):
                        nc.tensor.matmul(
                            ps[:, :w],
                            lhsT=wk_sb[:, dt_i, at * 128:(at + 1) * 128],
                            rhs=ikt_sc[sc][:, dt_i, :],
                            start=(dt_i == 0),
                            stop=(dt_i == NDT - 1),
                        )
                    psum_copy(kt_sb[:, at, sc * 384:sc * 384 + w], ps[:, :w])
                for sti in range(w // 128):
                    st = sc * 3 + sti
                    ps = scp.tile([128, HG, 64], f32, tag="mm")
                    for dt_i in range(NDT):
                        nc.tensor.matmul(
                            ps[:],
                            lhsT=ikt_sc[sc][:, dt_i,
                                            sti * 128:(sti + 1) * 128],
                            rhs=wv_sb[:, dt_i, :],
                            start=(dt_i == 0),
                            stop=(dt_i == NDT - 1),
                        )
                    psum_copy(v_sb[:, st, :, 0:64], ps[:])

            # ---- Q projection chunk 0 (chunk 1 is attention filler) ----
            def emit_qproj(at, qc, c, eng="flip"):
                # one 512-col slice of qT: 8 accumulating matmuls + copy
                ps = scp.tile([128, 512], f32, tag="mm")
                for dt_i in range(NDT):
                    nc.tensor.matmul(
                        ps[:],
                        lhsT=wq_sb[:, dt_i, at * 128:(at + 1) * 128],
                        rhs=iqt_c[qc][:, dt_i, c:c + 512],
                        start=(dt_i == 0),
                        stop=(dt_i == NDT - 1),
                    )
                dst = qt_c[qc][:, at, c:c + 512]
                if eng == "flip":
                    psum_copy(dst, ps[:])
                else:
                    nc.vector.tensor_copy(out=dst, in_=ps[:])

            for at in range(2):
                for c in (0, 512):
                    emit_qproj(at, 0, c)
            for at in range(2):
                for c in (0, 512):
                    emit_qproj(at, 1, c)

            # ---- Wo for one 128-row q tile ----
            def emit_wo(qt, copy_on_act=False):
                ps = scp.tile([128, 1024], f32, tag="mm")
                for c in range(2):
                    for t in range(2):
                        nc.tensor.matmul(
                            ps[:, c * 512:(c + 1) * 512],
                            lhsT=ctxn[qt // 8][t][:, (qt % 8) * 128:
                                                  (qt % 8 + 1) * 128],
                            rhs=wo_sb[:, t, c * 512:(c + 1) * 512],
                            start=(t == 0),
                            stop=(t == 1),
                        )
                ob = outp.tile([128, 1024], bf16, tag="ob")
                if copy_on_act:
                    nc.scalar.activation(out=ob[:], in_=ps[:], func=COPY)
                else:
                    nc.vector.tensor_copy(out=ob[:], in_=ps[:])
                nc.sync.dma_start(out[:, qt, :], ob[:])

            # ---- attention: per (q-chunk, head) with PE fillers ----
            # fillers[qc][h] emitted right after head h's norm chain
            def fill_qproj(at, c):
                # DVE copy: mid-attention the ACT engine is the pacer
                return lambda: emit_qproj(at, 1, c, eng="dve")

            def fill_wo(qt):
                return lambda: emit_wo(qt)

            # no mid-attention fillers: sustained all-engine activity trips
            # the HAM power clamp (k=4/8 duty for ~25us windows); the PE
            # idle at each head boundary is the power release valve.
            fillers = {}

            # Batched normalize: per head, copy ctx (rows 0..64) to SBUF --
            # frees the PSUM bank fast -- and gather the denominator row
            # onto partition h of a small tile via an SBUF->SBUF DMA.  One
            # exact DVE reciprocal then serves several heads at once (DVE
            # cost depends only on the free-dim size, so [4,1024] costs the
            # same 6.5us as [1,1024]); this frees ~3.6us/head of DVE duty,
            # keeping total engine power under the HAM clamp threshold.
            # shared across both q-chunks: qc1's writes naturally wait for
            # qc0's (long-finished) reads
            ctxu4 = persist.tile([65, HG, 1024], f32, tag="ctxu4")
            den4 = persist.tile([HG, 1024], f32, tag="den4")
            recip4 = persist.tile([HG, 1024], f32, tag="recip4")

            def emit_norm_batch(p_qc, heads):
                # reciprocal for several heads at once, then per-head
                # broadcast + multiply into the packed ctxn tiles
                nc.vector.reciprocal(
                    recip4[heads[0]:heads[-1] + 1, :],
                    den4[heads[0]:heads[-1] + 1, :],
                )
                for h in heads:
                    if h == 0:
                        rsrc = recip4[0:1, :]
                    else:
                        # gpsimd APs must start at partition 0: bounce the
                        # row down via a tiny SBUF->SBUF DMA
                        rs = rsp.tile([1, 1024], f32, tag="rs")
                        nc.sync.dma_start(rs[:], recip4[h:h + 1, :])
                        rsrc = rs[:]
                    bcd = bcp.tile([64, 1024], f32, tag="bcd")
                    nc.gpsimd.partition_broadcast(bcd[:], rsrc)
                    nc.vector.tensor_tensor(
                        ctxn[p_qc][h // 2][(h % 2) * 64:(h % 2) * 64 + 64, :],
                        ctxu4[0:64, h, :], bcd[:], MULT
                    )

            last_ctx = None
            for qc in range(2):
                for h in range(HG):
                    po = (h % 2) * 64
                    ti = h // 2
                    ctx = ctxp.tile([128, 1024], f32, tag="ctx")
                    for st in range(nst):
                        sc = scp.tile([128, 1024], f32, tag="mm")
                        for c in range(2):
                            nc.tensor.matmul(
                                sc[:, c * 512:(c + 1) * 512],
                                lhsT=kt_sb[po:po + 64, ti, st * 128:(st + 1) * 128],
                                rhs=qt_c[qc][po:po + 64, ti,
                                             c * 512:(c + 1) * 512],
                                start=True,
                                stop=True,
                            )
                        ex = expp.tile([128, 1024], bf16, tag="exp")
                        nc.scalar.activation(
                            out=ex[:], in_=sc[:], func=EXP,
                            bias=mb_sb[:, st:st + 1], scale=0.125,
                        )
                        for c in range(2):
                            nc.tensor.matmul(
                                ctx[:, c * 512:(c + 1) * 512],
                                lhsT=v_sb[:, st, h, :],
                                rhs=ex[:, c * 512:(c + 1) * 512],
                                start=(st == 0),
                                stop=(st == nst - 1),
                            )
                    if (qc, h) == (1, HG - 1):
                        # tail head: normalized straight from PSUM with the
                        # reciprocal on ACT (idle at the tail)
                        last_ctx = ctx
                        continue
                    nc.vector.tensor_copy(
                        out=ctxu4[:, h, :], in_=ctx[:65, :]
                    )
                    if (qc, h) == (0, HG - 1):
                        # one DMA gathers all 4 denominator rows onto
                        # partitions 0..3 (partition-crossing reshape)
                        nc.sync.dma_start(
                            den4[0:4, :], ctxu4[64:65, 0:4, :]
                        )
                        emit_norm_batch(0, [0, 1, 2, 3])
                    elif (qc, h) == (1, HG - 2):
                        nc.sync.dma_start(
                            den4[0:3, :], ctxu4[64:65, 0:3, :]
                        )
                        emit_norm_batch(1, [0, 1, 2])

            # tail: last head's normalize (ACT ln->exp) + all 16 Wo tiles
            lnd = srp.tile([1, 1024], f32, tag="lnd")
            nc.scalar.activation(out=lnd[:], in_=last_ctx[64:65, :], func=LN)
            recip = srp.tile([1, 1024], f32, tag="recip")
            nc.scalar.activation(out=recip[:], in_=lnd[:], func=EXP,
                                 scale=-1.0)
            bcd = bcp.tile([64, 1024], f32, tag="bcd")
            nc.gpsimd.partition_broadcast(bcd[:], recip[:])
            nc.vector.tensor_tensor(
                ctxn[1][1][64:128, :], last_ctx[0:64, :], bcd[:], MULT
            )
            for i, qt in enumerate(range(16)):
                emit_wo(qt, copy_on_act=(i % 2 == 0))

    nc.compile()
    return nc


def _get_program(nst):
    if nst not in _cache:
        _cache[nst] = _build_program(nst)
    return _cache[nst]


def _prep_inputs(iQ, iK, mask, Wq, Wkv, Wo):
    """Build the 8 per-core input maps (host-side shard + prune + cast)."""
    bf = ml_dtypes.bfloat16
    iQ = np.asarray(iQ, dtype=np.float32)
    iK = np.asarray(iK, dtype=np.float32)
    mask = np.asarray(mask)
    Wq = np.asarray(Wq, dtype=np.float32)
    Wkv = np.asarray(Wkv, dtype=np.float32)
    Wo = np.asarray(Wo, dtype=np.float32)

    def tile_kxn(a):  # [K=1024, N] -> [128, K/128, N]
        K, N = a.shape
        return np.ascontiguousarray(
            a.reshape(K // 128, 128, N).transpose(1, 0, 2)
        )

    kept = [np.flatnonzero(~mask[b, 0]) for b in range(B)]
    nst = max(MIN_NST, max((len(k) + 127) // 128 for k in kept))
    nst = ((nst + 2) // 3) * 3  # chunked ikt streaming wants 3 tiles/chunk
    SP = nst * 128

    per_b = {}
    for b in range(B):
        nk = len(kept[b])
        ikt_full = np.zeros((1024, SP), dtype=np.float32)
        ikt_full[:, :nk] = iK[b][kept[b], :].T
        bias = np.full(SP, np.float32(NEG), dtype=np.float32)
        bias[:nk] = 0.0
        ikt_t = tile_kxn(ikt_full).astype(bf)  # [128, 8, SP]
        per_b[b] = {
            "iqt": tile_kxn(iQ[b].T).astype(bf),
            # chunk-major [128, NSC, 8, 384] for contiguous chunk DMAs
            "ikt": np.ascontiguousarray(
                ikt_t.reshape(128, 8, nst // 3, 384).transpose(0, 2, 1, 3)
            ),
            "mb": np.ascontiguousarray(bias.reshape(nst, 128).T),
        }
    in_maps = []
    for c in range(NCORES):
        b, g = divmod(c, NCORES // B)
        cols = slice(g * 256, (g + 1) * 256)
        wo_g = Wo[g * 256:(g + 1) * 256, :]          # [256, 1024]
        in_maps.append({
            "iqt": per_b[b]["iqt"],
            "ikt": per_b[b]["ikt"],
            "mb": per_b[b]["mb"],
            "wq": tile_kxn(Wq[:, cols]).astype(bf),
            "wk": tile_kxn(Wkv[:, cols]).astype(bf),
            "wv": tile_kxn(Wkv[:, 1024 + g * 256:1024 + (g + 1) * 256]).astype(bf),
            "wo": np.ascontiguousarray(
                wo_g.reshape(2, 128, D).transpose(1, 0, 2)
            ).astype(bf),
        })
    return in_maps, nst


def _run(inputs, trace=False):
    from concourse.bass_utils import run_bass_kernel_spmd

    in_maps, nst = _prep_inputs(**inputs)
    nc = _get_program(nst)
    res = run_bass_kernel_spmd(
        nc, in_maps, list(range(NCORES)), trace=trace
    )
    outs = []
    for b in range(B):
        acc = None
        for g in range(NCORES // B):
            o = np.asarray(
                res.results[b * (NCORES // B) + g]["out"], dtype=np.float32
            )
            acc = o if acc is None else acc + o
        # [128, 16, 1024] -> [2048, 1024]
        outs.append(acc.transpose(1, 0, 2).reshape(Q, D))
    return np.stack(outs), res


def kernel(**inputs):
    out, _ = _run(inputs, trace=False)
    return out


# revision 57
# speedup vs baseline: 1.0727x; 1.0727x over previous
"""CrossAttention Trainium2 Bass kernel.

Problem: B=2, Q=S=2048, D=1024, H=16 heads, A=64 head_dim.
  q = (iQ @ Wq)   -> [B,H,Q,A]
  k,v = iK @ Wkv  -> [B,H,S,A] each
  scores = q k^T / 8, mask -> -1e9, softmax over S
  out = (attn @ v) @ Wo -> [B,Q,D]

Sharding: 8 cores = 2 batches x 4 head-groups (4 heads each).
Each core computes a partial [Q, D] = ctx_local @ Wo_rows(local heads);
host sums the 4 partials per batch (row-parallel Wo unshard).

Mask pruning: masked s positions contribute exactly 0 to the softmax
(exp(-1e9) == 0.0 in f32, same as the reference), so the host gathers
only the unmasked iK rows (~S/2 of them), padded to a multiple of 128
with zero K columns and -1e9 bias so padding also exps to exactly 0.

Device layout trick: everything is computed "transposed" (feature dim on
partitions) so no on-device transposes are needed:
  - host ships iQ^T, iK^T (pre-tiled [128, 8, n])
  - qT[a,q], kT[a,s] from matmul(lhsT=W, rhs=iX^T)
  - scoresT[s,q] = matmul(lhsT=kT_slice, rhs=qT)       (K=64 contraction)
  - exp via scalar activation, mask bias is a per-partition bias AP
  - V kept natural [s,a] with an appended ones column -> attn@V matmul
    also yields the softmax denominator row for free

Schedule (the attention phase is paced by the ACT engine's exps --
72 x ~1.1us -- and the PE's p-state only ramps to full clock under
continuous execution, so the PE stream is kept dense):
  - DMA order = consumption order: wk, ikt, wq, wv, mb, iqt(c0),
    iqt(c1), wo.  iqt is split into two SBUF tiles so chunk-0 Q proj
    doesn't wait on the chunk-1 transfer.
  - PE: K proj -> V proj -> Q proj(c0) -> attention qc0 (Q proj c1
    matmuls spread between heads as filler) -> attention qc1 (Wo for
    qc0's 8 q-tiles spread between heads) -> tail Wo for qc1.
  - normalize: reciprocal_approx_fast on DVE (1.3us, vs 6.5us exact),
    gpsimd partition_broadcast, DVE multiply straight out of PSUM.
    ctx PSUM is double-buffered so the next head never waits on it.
  - Wo psum->sbuf copies run on DVE mid-phase (ACT is the pacer there)
    and alternate ACT/DVE in the tail (ACT is idle there).
"""

import sys
import numpy as np

for _p in ("/opt/trn_rl_repo",):
    if _p not in sys.path:
        sys.path.insert(0, _p)

import ml_dtypes

B, Q, S, D = 2, 2048, 2048, 1024
H, A = 16, 64
HG = 4            # heads per core
NCORES = 8
NEG = -1e9
MIN_NST = 9       # S tiles after mask pruning (1152 slots; count ~1024)

_cache = {}


def _build_program(nst):
    import concourse.bass as bass  # noqa
    import concourse.bacc as bacc
    import concourse.tile as tile
    from concourse import mybir

    f32 = mybir.dt.float32
    bf16 = mybir.dt.bfloat16
    EXP = mybir.ActivationFunctionType.Exp
    COPY = mybir.ActivationFunctionType.Copy
    LN = mybir.ActivationFunctionType.Ln
    MULT = mybir.AluOpType.mult

    nc = bacc.Bacc("TRN2", target_bir_lowering=False, debug=False)

    SP = nst * 128  # padded kept-S extent (nst is a multiple of 3)
    NSC_ = nst // 3
    iqt = nc.dram_tensor("iqt", [128, 8, Q], bf16, kind="ExternalInput").ap()
    # chunk-major so each s-chunk is one contiguous-per-partition transfer
    ikt = nc.dram_tensor(
        "ikt", [128, NSC_, 8, 384], bf16, kind="ExternalInput"
    ).ap()
    wq = nc.dram_tensor("wq", [128, 8, 256], bf16, kind="ExternalInput").ap()
    wk = nc.dram_tensor("wk", [128, 8, 256], bf16, kind="ExternalInput").ap()
    wv = nc.dram_tensor("wv", [128, 8, 256], bf16, kind="ExternalInput").ap()
    wo = nc.dram_tensor("wo", [128, 2, D], bf16, kind="ExternalInput").ap()
    mb = nc.dram_tensor("mb", [128, nst], f32, kind="ExternalInput").ap()
    out = nc.dram_tensor("out", [128, 16, D], bf16, kind="ExternalOutput").ap()

    NDT = D // 128          # 8 d tiles

    with tile.TileContext(nc) as tc:
        with (
            tc.tile_pool(name="persist", bufs=1) as persist,
            tc.tile_pool(name="expp", bufs=4) as expp,
            tc.tile_pool(name="outp", bufs=3) as outp,
            tc.tile_pool(name="srp", bufs=1) as srp,
            tc.tile_pool(name="bcp", bufs=3) as bcp,
            tc.tile_pool(name="rsp", bufs=2) as rsp,
            tc.tile_pool(name="scp", bufs=2, space="PSUM") as scp,
            tc.tile_pool(name="ctxp", bufs=2, space="PSUM") as ctxp,
        ):
            # ---- persistent tiles ----
            wk_sb = persist.tile([128, 8, 256], bf16, tag="wk")
            wq_sb = persist.tile([128, 8, 256], bf16, tag="wq")
            wv_sb = persist.tile([128, 8, 256], bf16, tag="wv")
            wo_sb = persist.tile([128, 2, D], bf16, tag="wo")
            # ikt split into s-major chunks (3 s-tiles each) so the K/V
            # projections start as soon as the first chunk lands instead of
            # waiting for the whole 2.25MB transfer.
            NSC = nst // 3
            ikt_sc = [
                persist.tile(
                    [128, 8, 384], bf16, tag=f"ikt{sc}", name=f"ikt{sc}",
                )
                for sc in range(NSC)
            ]
            # iqt split per q-chunk so chunk-0 consumers don't wait on the
            # chunk-1 DMA (tile-granular dependency tracking).
            iqt_c = [
                persist.tile([128, 8, 1024], bf16, tag=f"iqt{c}", name=f"iqt{c}")
                for c in range(2)
            ]
            mb_sb = persist.tile([128, nst], f32, tag="mb")
            # qt split per q-chunk: chunk-1 projection lands mid-attention
            # and must not create a (tile-granular) false dependency for
            # chunk-0 score reads.
            qt_c = [
                persist.tile([128, 2, 1024], bf16, tag=f"qt{c}", name=f"qt{c}")
                for c in range(2)
            ]
            kt_sb = persist.tile([128, 2, SP], bf16, tag="kt")
            # V padded to 128 cols (fast weight load wants full-width lhsT);
            # col 64 = ones (softmax denominator row), cols 65.. = zeros.
            v_sb = persist.tile([128, nst, HG, 128], bf16, tag="v")
            # two heads packed per 128-partition tile for the Wo matmul;
            # separate tiles per q-chunk so Wo reads of chunk 0 don't
            # falsely depend on chunk 1 writes.
            # split into two 512-col q-halves per (qc, t) so the tail
            # head's first normalized half releases Wo tiles early
            ctxn = [
                [
                    [
                        persist.tile(
                            [128, 512], bf16, tag=f"ctxn{qc}{t}{hf}",
                            name=f"ctxn{qc}{t}{hf}"
                        )
                        for hf in range(2)
                    ]
                    for t in range(2)
                ]
                for qc in range(2)
            ]

            # ---- DMAs in consumption order, consolidated (one issue per
            # tensor chunk costs ~620ns of queue time) and split across
            # the two HWDGE-capable queues (sync + scalar).
            nc.scalar.dma_start(wk_sb[:], wk[:])
            nc.scalar.dma_start(wv_sb[:], wv[:])
            for sc in range(NSC):
                nc.sync.dma_start(ikt_sc[sc][:], ikt[:, sc, :, :])
            nc.scalar.dma_start(wq_sb[:], wq[:])
            nc.scalar.dma_start(mb_sb[:], mb[:])
            for c in range(2):
                nc.scalar.dma_start(
                    iqt_c[c][:], iqt[:, :, c * 1024:(c + 1) * 1024]
                )
            nc.sync.dma_start(wo_sb[:], wo[:])

            # v_sb zero/ones init on gpsimd (idle early; DVE stays free).
            nc.gpsimd.memset(v_sb[:], 0.0)
            nc.gpsimd.memset(v_sb[:, :, :, 64:65], 1.0)

            # ---- K projection: kT [a, s], both 128-row at-slabs ----
            # copies alternate ACT/DVE so neither serializes the chain.
            cp_flip = [0]

            def psum_copy(dst, src):
                if cp_flip[0] % 2 == 0:
                    nc.vector.tensor_copy(out=dst, in_=src)
                else:
                    nc.scalar.activation(out=dst, in_=src, func=COPY)
                cp_flip[0] += 1

            # K and V projections interleaved per s-chunk: each chunk's
            # matmuls start as soon as its 8 ikt d-tiles land.
            for sc in range(NSC):
                w = ikt_sc[sc].shape[2]
                for at in range(2):
                    ps = scp.tile([128, 384], f32, tag="mm")
                    for dt_i in range(NDT):
                        nc.tensor.matmul(
                            ps[:, :w],
                            lhsT=wk_sb[:, dt_i, at * 128:(at + 1) * 128],
                            rhs=ikt_sc[sc][:, dt_i, :],
                            start=(dt_i == 0),
                            stop=(dt_i == NDT - 1),
                        )
                    psum_copy(kt_sb[:, at, sc * 384:sc * 384 + w], ps[:, :w])
                for sti in range(w // 128):
                    st = sc * 3 + sti
                    ps = scp.tile([128, HG, 64], f32, tag="mm")
                    for dt_i in range(NDT):
                        nc.tensor.matmul(
                            ps[:],
                            lhsT=ikt_sc[sc][:, dt_i,
                                            sti * 128:(sti + 1) * 128],
                            rhs=wv_sb[:, dt_i, :],
                            start=(dt_i == 0),
                            stop=(dt_i == NDT - 1),
                        )
                    psum_copy(v_sb[:, st, :, 0:64], ps[:])

            # ---- Q projection chunk 0 (chunk 1 is attention filler) ----
            def emit_qproj(at, qc, c, eng="flip"):
                # one 512-col slice of qT: 8 accumulating matmuls + copy
                ps = scp.tile([128, 512], f32, tag="mm")
                for dt_i in range(NDT):
                    nc.tensor.matmul(
                        ps[:],
                        lhsT=wq_sb[:, dt_i, at * 128:(at + 1) * 128],
                        rhs=iqt_c[qc][:, dt_i, c:c + 512],
                        start=(dt_i == 0),
                        stop=(dt_i == NDT - 1),
                    )
                dst = qt_c[qc][:, at, c:c + 512]
                if eng == "flip":
                    psum_copy(dst, ps[:])
                else:
                    nc.vector.tensor_copy(out=dst, in_=ps[:])

            for at in range(2):
                for c in (0, 512):
                    emit_qproj(at, 0, c)
            for at in range(2):
                for c in (0, 512):
                    emit_qproj(at, 1, c)

            # ---- Wo for one 128-row q tile ----
            def emit_wo(qt, copy_on_act=False):
                ps = scp.tile([128, 1024], f32, tag="mm")
                for c in range(2):
                    for t in range(2):
                        nc.tensor.matmul(
                            ps[:, c * 512:(c + 1) * 512],
                            lhsT=ctxn[qt // 8][t][(qt % 8) // 4][
                                :, (qt % 4) * 128:(qt % 4 + 1) * 128],
                            rhs=wo_sb[:, t, c * 512:(c + 1) * 512],
                            start=(t == 0),
                            stop=(t == 1),
                        )
                ob = outp.tile([128, 1024], bf16, tag="ob")
                if copy_on_act:
                    nc.scalar.activation(out=ob[:], in_=ps[:], func=COPY)
                else:
                    nc.vector.tensor_copy(out=ob[:], in_=ps[:])
                nc.sync.dma_start(out[:, qt, :], ob[:])

            # ---- attention: per (q-chunk, head) with PE fillers ----
            # fillers[qc][h] emitted right after head h's norm chain
            def fill_qproj(at, c):
                # DVE copy: mid-attention the ACT engine is the pacer
                return lambda: emit_qproj(at, 1, c, eng="dve")

            def fill_wo(qt):
                return lambda: emit_wo(qt)

            # no mid-attention fillers: sustained all-engine activity trips
            # the HAM power clamp (k=4/8 duty for ~25us windows); the PE
            # idle at each head boundary is the power release valve.
            fillers = {}

            # Batched normalize: per head, copy ctx (rows 0..64) to SBUF --
            # frees the PSUM bank fast -- and gather the denominator row
            # onto partition h of a small tile via an SBUF->SBUF DMA.  One
            # exact DVE reciprocal then serves several heads at once (DVE
            # cost depends only on the free-dim size, so [4,1024] costs the
            # same 6.5us as [1,1024]); this frees ~3.6us/head of DVE duty,
            # keeping total engine power under the HAM clamp threshold.
            # shared across both q-chunks: qc1's writes naturally wait for
            # qc0's (long-finished) reads
            ctxu4 = persist.tile([65, HG, 1024], f32, tag="ctxu4")
            den4 = persist.tile([HG, 1024], f32, tag="den4")
            recip4 = persist.tile([HG, 1024], f32, tag="recip4")

            def emit_norm_batch(p_qc, heads):
                # reciprocal for several heads at once, then per-head
                # broadcast + multiply into the packed ctxn tiles
                nc.vector.reciprocal(
                    recip4[heads[0]:heads[-1] + 1, :],
                    den4[heads[0]:heads[-1] + 1, :],
                )
                for h in heads:
                    if h == 0:
                        rsrc = recip4[0:1, :]
                    else:
                        # gpsimd APs must start at partition 0: bounce the
                        # row down via a tiny SBUF->SBUF DMA
                        rs = rsp.tile([1, 1024], f32, tag="rs")
                        nc.sync.dma_start(rs[:], recip4[h:h + 1, :])
                        rsrc = rs[:]
                    bcd = bcp.tile([64, 1024], f32, tag="bcd")
                    nc.gpsimd.partition_broadcast(bcd[:], rsrc)
                    for hf in range(2):
                        nc.vector.tensor_tensor(
                            ctxn[p_qc][h // 2][hf][
                                (h % 2) * 64:(h % 2) * 64 + 64, :],
                            ctxu4[0:64, h, hf * 512:(hf + 1) * 512],
                            bcd[:, hf * 512:(hf + 1) * 512], MULT
                        )

            last_ctx = None
            for qc in range(2):
                for h in range(HG):
                    po = (h % 2) * 64
                    ti = h // 2
                    ctx = ctxp.tile([128, 1024], f32, tag="ctx")
                    for st in range(nst):
                        sc = scp.tile([128, 1024], f32, tag="mm")
                        for c in range(2):
                            nc.tensor.matmul(
                                sc[:, c * 512:(c + 1) * 512],
                                lhsT=kt_sb[po:po + 64, ti, st * 128:(st + 1) * 128],
                                rhs=qt_c[qc][po:po + 64, ti,
                                             c * 512:(c + 1) * 512],
                                start=True,
                                stop=True,
                            )
                        ex = expp.tile([128, 1024], bf16, tag="exp")
                        nc.scalar.activation(
                            out=ex[:], in_=sc[:], func=EXP,
                            bias=mb_sb[:, st:st + 1], scale=0.125,
                        )
                        for c in range(2):
                            nc.tensor.matmul(
                                ctx[:, c * 512:(c + 1) * 512],
                                lhsT=v_sb[:, st, h, :],
                                rhs=ex[:, c * 512:(c + 1) * 512],
                                start=(st == 0),
                                stop=(st == nst - 1),
                            )
                    if (qc, h) == (1, HG - 1):
                        # tail head: normalized straight from PSUM with the
                        # reciprocal on ACT (idle at the tail)
                        last_ctx = ctx
                        continue
                    nc.vector.tensor_copy(
                        out=ctxu4[:, h, :], in_=ctx[:65, :]
                    )
                    if (qc, h) == (0, HG - 1):
                        # one DMA gathers all 4 denominator rows onto
                        # partitions 0..3 (partition-crossing reshape)
                        nc.sync.dma_start(
                            den4[0:4, :], ctxu4[64:65, 0:4, :]
                        )
                        emit_norm_batch(0, [0, 1, 2, 3])
                    elif (qc, h) == (1, HG - 2):
                        nc.sync.dma_start(
                            den4[0:3, :], ctxu4[64:65, 0:3, :]
                        )
                        emit_norm_batch(1, [0, 1, 2])

            # tail: last head's normalize per q-half (ACT ln->exp; the
            # first half releases Wo qt8-11 early) + all 16 Wo tiles
            for i, qt in enumerate(range(8)):
                emit_wo(qt, copy_on_act=(i % 2 == 0))
            for hf in range(2):
                sl = slice(hf * 512, (hf + 1) * 512)
                lnd = srp.tile([1, 512], f32, tag="lnd")
                nc.scalar.activation(out=lnd[:], in_=last_ctx[64:65, sl],
                                     func=LN)
                recip = srp.tile([1, 512], f32, tag="recip")
                nc.scalar.activation(out=recip[:], in_=lnd[:], func=EXP,
                                     scale=-1.0)
                bcd = bcp.tile([64, 512], f32, tag="bcd")
                nc.gpsimd.partition_broadcast(bcd[:], recip[:])
                nc.vector.tensor_tensor(
                    ctxn[1][1][hf][64:128, :], last_ctx[0:64, sl], bcd[:],
                    MULT
                )
                for i, qt in enumerate(range(8 + hf * 4, 12 + hf * 4)):
                    emit_wo(qt, copy_on_act=(i % 2 == 0))

    nc.compile()
    return nc


def _get_program(nst):
    if nst not in _cache:
        _cache[nst] = _build_program(nst)
    return _cache[nst]


def _prep_inputs(iQ, iK, mask, Wq, Wkv, Wo):
    """Build the 8 per-core input maps (host-side shard + prune + cast)."""
    bf = ml_dtypes.bfloat16
    iQ = np.asarray(iQ, dtype=np.float32)
    iK = np.asarray(iK, dtype=np.float32)
    mask = np.asarray(mask)
    Wq = np.asarray(Wq, dtype=np.float32)
    Wkv = np.asarray(Wkv, dtype=np.float32)
    Wo = np.asarray(Wo, dtype=np.float32)

    def tile_kxn(a):  # [K=1024, N] -> [128, K/128, N]
        K, N = a.shape
        return np.ascontiguousarray(
            a.reshape(K // 128, 128, N).transpose(1, 0, 2)
        )

    kept = [np.flatnonzero(~mask[b, 0]) for b in range(B)]
    nst = max(MIN_NST, max((len(k) + 127) // 128 for k in kept))
    nst = ((nst + 2) // 3) * 3  # chunked ikt streaming wants 3 tiles/chunk
    SP = nst * 128

    per_b = {}
    for b in range(B):
        nk = len(kept[b])
        ikt_full = np.zeros((1024, SP), dtype=np.float32)
        ikt_full[:, :nk] = iK[b][kept[b], :].T
        bias = np.full(SP, np.float32(NEG), dtype=np.float32)
        bias[:nk] = 0.0
        ikt_t = tile_kxn(ikt_full).astype(bf)  # [128, 8, SP]
        per_b[b] = {
            "iqt": tile_kxn(iQ[b].T).astype(bf),
            # chunk-major [128, NSC, 8, 384] for contiguous chunk DMAs
            "ikt": np.ascontiguousarray(
                ikt_t.reshape(128, 8, nst // 3, 384).transpose(0, 2, 1, 3)
            ),
            "mb": np.ascontiguousarray(bias.reshape(nst, 128).T),
        }
    in_maps = []
    for c in range(NCORES):
        b, g = divmod(c, NCORES // B)
        cols = slice(g * 256, (g + 1) * 256)
        wo_g = Wo[g * 256:(g + 1) * 256, :]          # [256, 1024]
        in_maps.append({
            "iqt": per_b[b]["iqt"],
            "ikt": per_b[b]["ikt"],
            "mb": per_b[b]["mb"],
            "wq": tile_kxn(Wq[:, cols]).astype(bf),
            "wk": tile_kxn(Wkv[:, cols]).astype(bf),
            "wv": tile_kxn(Wkv[:, 1024 + g * 256:1024 + (g + 1) * 256]).astype(bf),
            "wo": np.ascontiguousarray(
                wo_g.reshape(2, 128, D).transpose(1, 0, 2)
            ).astype(bf),
        })
    return in_maps, nst


def _run(inputs, trace=False):
    from concourse.bass_utils import run_bass_kernel_spmd

    in_maps, nst = _prep_inputs(**inputs)
    nc = _get_program(nst)
    res = run_bass_kernel_spmd(
        nc, in_maps, list(range(NCORES)), trace=trace
    )
    outs = []
    for b in range(B):
        acc = None
        for g in range(NCORES // B):
            o = np.asarray(
                res.results[b * (NCORES // B) + g]["out"], dtype=np.float32
            )
            acc = o if acc is None else acc + o
        # [128, 16, 1024] -> [2048, 1024]
        outs.append(acc.transpose(1, 0, 2).reshape(Q, D))
    return np.stack(outs), res


def kernel(**inputs):
    out, _ = _run(inputs, trace=False)
    return out
